# revision 18
# baseline (speedup 1.0000x reference)
"""CoarseMatching Trainium2 kernel: 8-core SPMD, row-sharded dual-softmax.

Strategy: shard the 6400-row L dim of sim12/sim14 (and 1600 of sim18) across
8 cores; replicate the 400-row 1/16 scale and the tiny t-pyramid. Two passes
per big matrix: pass 1 (transposed orientation) computes exp-sum statistics
(column sums via ACT accum, row sums via bf16 ones-matmul); one small
AllReduce combines column sums; pass 2 recomputes sims (f32r full-rate
matmuls) and emits outputs / argmaxes. Pool partials go through one small
AllGather. All heavy matmuls run in float32r (tf32-class, ~1e-4 rel err).
"""
import sys
sys.path.insert(0, '/opt/trn_rl_repo')
import numpy as np

N_CORES = 8
SH12 = 800    # rows of feats12_0 / feats14_0 per core
SH18 = 200    # rows of feats18_0 per core
C12, C14, C18, C116 = 45.0, 60.0, 70.0, 55.0
LNT = float(np.log(0.1))

_nc_cache = None


def _build_nc():
    import concourse.bass as bass
    import concourse.bacc as bacc
    import concourse.mybir as mybir
    import concourse.tile as tile
    from concourse import library_config

    F32, F32R, BF16 = mybir.dt.float32, mybir.dt.float32r, mybir.dt.bfloat16
    U16, I16, U32, I32 = (mybir.dt.uint16, mybir.dt.int16, mybir.dt.uint32,
                          mybir.dt.int32)
    AF = mybir.ActivationFunctionType
    ALU = mybir.AluOpType
    AP = bass.AP

    nc = bacc.Bacc("TRN2", target_bir_lowering=False, debug=False,
                   num_devices=N_CORES)

    def inp(name, shape, dt=F32R):
        return nc.dram_tensor(name, shape, dt, kind="ExternalInput")

    def inp_f32(name, shape):
        return nc.dram_tensor(name, shape, F32, kind="ExternalInput")

    f12_1T = inp("f12_1T", [128, 6400])
    f12catT = inp("f12catT", [128, 1600])
    f14_0T_hi = inp("f14_0T_hi", [128, 800])
    f14_0T_lo = inp("f14_0T_lo", [80, 800])     # row 68 = ones
    f14_1T_hi = inp("f14_1T_hi", [128, 6400])
    f14_1T_lo = inp("f14_1T_lo", [128, 8576])   # row 68 aug14; row 96 aug12/18/116
    f18_0T_hi = inp("f18_0T_hi", [128, 256])    # cols 200:256 zero pad
    f18_0T_lo = inp("f18_0T_lo", [128, 256])
    f18_1T_hi = inp("f18_1T_hi", [128, 1600])
    f18_1T_lo = inp("f18_1T_lo", [128, 1600])
    f116_0T_hi = inp("f116_0T_hi", [128, 400])
    f116_0T_lo = inp("f116_0T_lo", [128, 400])
    f116_1T_hi = inp("f116_1T_hi", [128, 400])
    f116_1T_lo = inp("f116_1T_lo", [128, 400])
    t2T_hi = inp("t2T_hi", [128, 400])
    t2T_lo = inp("t2T_lo", [128, 400])
    t3T_hi = inp_f32("t3T_hi", [128, 256])          # cols 100:256 zero pad
    t3T_lo = inp_f32("t3T_lo", [68, 256])
    t4T = inp_f32("t4T", [128, 32])
    W116T = inp_f32("W116T", [768, 256])
    W18T = inp_f32("W18T", [512, 256])
    Wt3T = inp_f32("Wt3T", [1100, 128])
    Wt4T = inp_f32("Wt4T", [256, 128])
    ones1r = inp("ones1r", [1, 128])
    onesb_d = inp("onesb", [128, 1], BF16)

    conf18_o = nc.dram_tensor("conf18_o", [200, 1600], F32, kind="ExternalOutput")
    conf12_o = nc.dram_tensor("conf12_o", [800, 6400], F32, kind="ExternalOutput")
    topic_o = nc.dram_tensor("topic_o", [1600], I32, kind="ExternalOutput")

    # DRAM scratch (f32) for partition-major <-> free-major rearranges
    b12d = nc.dram_tensor("b12d", [800], F32)
    b18d = nc.dram_tensor("b18d", [256], F32)
    aug12d = nc.dram_tensor("aug12d", [6400], F32)
    aug14d = nc.dram_tensor("aug14d", [6400], F32)
    aug18d = nc.dram_tensor("aug18d", [1664], F32)
    aug116d = nc.dram_tensor("aug116d", [512], F32)
    i14d = nc.dram_tensor("i14d", [896], I16)
    i18d = nc.dram_tensor("i18d", [256], I16)
    i116d = nc.dram_tensor("i116d", [512], I16)
    t2pd = nc.dram_tensor("t2pd", [2, 128, 256], F32)   # cast staging
    f18gd = nc.dram_tensor("f18gd", [2, 128, 256], F32)
    f116gd = nc.dram_tensor("f116gd", [2, 128, 400], F32)
    poold = nc.dram_tensor("poold", [6, 128, 256], F32)
    t3dd = nc.dram_tensor("t3dd", [128, 32], F32)
    t4sd = nc.dram_tensor("t4sd", [128, 32], F32)

    RG = [list(range(N_CORES))]

    with tile.TileContext(nc) as tc:
        with tc.tile_pool(name="cst", bufs=1) as cst, \
             tc.tile_pool(name="big", bufs=2) as big, \
             tc.tile_pool(name="work", bufs=2) as work, \
             tc.tile_pool(name="texp", bufs=2) as texp_p, \
             tc.tile_pool(name="wpool", bufs=2) as wpool, \
             tc.tile_pool(name="psA", bufs=2, space="PSUM") as psA, \
             tc.tile_pool(name="psB", bufs=1, space="PSUM") as psB, \
             tc.tile_pool(name="dram", bufs=1, space="DRAM") as dpool:

            # ---- persistent SBUF tiles ----
            def ld(dram_t, shape, dt=F32R, name=None):
                t = cst.tile(shape, dt, tag=dram_t.name + "_s", name=dram_t.name + "_s")
                nc.sync.dma_start(t[:], dram_t[:])
                return t

            s12_1 = ld(f12_1T, [128, 6400])
            s12cat = ld(f12catT, [128, 1600])
            s14_0h = ld(f14_0T_hi, [128, 800])
            s14_0l = ld(f14_0T_lo, [80, 800])
            s14_1h = ld(f14_1T_hi, [128, 6400])
            s14_1l = ld(f14_1T_lo, [128, 8576])
            s18_0h = ld(f18_0T_hi, [128, 256])
            s18_0l = ld(f18_0T_lo, [128, 256])
            s18_1h = ld(f18_1T_hi, [128, 1600])
            s18_1l = ld(f18_1T_lo, [128, 1600])
            s116_0h = ld(f116_0T_hi, [128, 400])
            s116_0l = ld(f116_0T_lo, [128, 400])
            s116_1h = ld(f116_1T_hi, [128, 400])
            s116_1l = ld(f116_1T_lo, [128, 400])
            st2h = ld(t2T_hi, [128, 400])
            st2l = ld(t2T_lo, [128, 400])
            st3h = ld(t3T_hi, [128, 256], F32)
            st3l = ld(t3T_lo, [68, 256], F32)
            st4 = ld(t4T, [128, 32], F32)
            augrow12 = s14_1l[96:97, 0:6400]
            augrow18 = s14_1l[96:97, 6400:8064]
            augrow116 = s14_1l[96:97, 8064:8576]
            sob = ld(onesb_d, [128, 1], BF16)
            ones96t = cst.tile([128, 128], F32R, tag="ones96t")

            # accumulators / small persistents
            cs12 = cst.tile([128, 50], F32, tag="cs12")
            cs14 = cst.tile([128, 50], F32, tag="cs14")
            cs18 = cst.tile([128, 13], F32, tag="cs18")
            cs116 = cst.tile([128, 4], F32, tag="cs116")
            bias12 = cst.tile([128, 7], F32, tag="bias12")
            bias18 = cst.tile([128, 2], F32, tag="bias18")
            idx14 = cst.tile([128, 7], U16, tag="idx14")
            idx18 = cst.tile([128, 2], U16, tag="idx18")
            idx116 = cst.tile([128, 4], U16, tag="idx116")
            topicacc = cst.tile([128, 13], U32, tag="topicacc")
            idx14w = cst.tile([128, 50], I16, tag="idx14w")
            idx18w = cst.tile([128, 13], I16, tag="idx18w")
            idx116w = cst.tile([128, 25], I16, tag="idx116w")
            mx8 = cst.tile([128, 8], F32, tag="mx8")
            mi8_16 = cst.tile([128, 8], U16, tag="mi8_16")
            mi8_32 = cst.tile([128, 8], U32, tag="mi8_32")
            lnst = cst.tile([128, 50], F32, tag="lnst")
            augst = cst.tile([128, 50], F32, tag="augst")
            bC12 = cst.tile([128, 1], F32, tag="bC12")
            bC14 = cst.tile([128, 1], F32, tag="bC14")
            bC18 = cst.tile([128, 1], F32, tag="bC18")
            bC116 = cst.tile([128, 1], F32, tag="bC116")
            bZero = cst.tile([128, 1], F32, tag="bZero")
            # pyramid f32r tiles
            t2pr = [cst.tile([128, 256], F32, tag=f"t2pr{i}", name=f"t2pr{i}") for i in range(2)]
            f18gr = [cst.tile([128, 256], F32, tag=f"f18gr{i}", name=f"f18gr{i}") for i in range(2)]
            f116gr = [cst.tile([128, 400], F32, tag=f"f116gr{i}", name=f"f116gr{i}") for i in range(2)]
            poolr = [cst.tile([128, 256], F32, tag=f"poolr{i}", name=f"poolr{i}") for i in range(6)]
            t3df = cst.tile([128, 32], F32, tag="t3df")
            rbst = cst.tile([128, 100], F32, tag="rbst")
            pfst = cst.tile([128, 256], F32, tag="pfst")
            t4sf = cst.tile([128, 32], F32, tag="t4sf")

            # DRAM pool tiles for collectives
            cc1_in = dpool.tile([14464], F32)
            cc1_out = dpool.tile([14464], F32)
            cc2_in = dpool.tile([52000], F32)
            cc2_out = dpool.tile([416000], F32)

            gp = nc.gpsimd
            gp.load_library(library_config.ap_gather)
            nc.vector.memset(ones96t[:].bitcast(F32), 1.0)
            for t, v in [(bC12, -C12), (bC14, -C14), (bC18, -C18),
                         (bC116, -C116), (bZero, 0.0)]:
                nc.vector.memset(t[:], v)
            for t, v in [(cs18, 1.0), (cs116, 1.0), (idx14, 0), (idx18, 0),
                         (idx116, 0), (topicacc, 0)]:
                gp.memset(t[:], v)

            # ============ PASS 1 (transposed orientation) ============
            # sim12^T tiles [s_chunk(128), l(800)]; colsum via accum, rowsum
            # via bf16 ones-matmul accumulated in PSUM.
            rs12ps = psB.tile([1, 800], F32, tag="rsB")
            for sc in range(50):
                ps = psA.tile([128, 800], F32, tag="p1ps")
                te = texp_p.tile([128, 800], BF16, tag="texp")
                lhs = s12_1[:, sc * 128:(sc + 1) * 128]
                nc.tensor.matmul(ps[:, 0:512], lhs, s12cat[:, 0:512],
                                 start=True, stop=True)
                nc.tensor.matmul(ps[:, 512:800], lhs, s12cat[:, 512:800],
                                 start=True, stop=True)
                nc.scalar.activation(te[:], ps[:], AF.Exp, bias=bC12[:],
                                     scale=1.0, accum_out=cs12[:, sc:sc + 1])
                nc.tensor.matmul(rs12ps[:, 0:512], sob[:], te[:, 0:512],
                                 start=(sc == 0), stop=(sc == 49))
                nc.tensor.matmul(rs12ps[:, 512:800], sob[:], te[:, 512:800],
                                 start=(sc == 0), stop=(sc == 49))
            # bias12 = -ln(rowsum12) - C12 - LNT, rearranged to [128,7]
            vrow = work.tile([128, 1600], F32, tag="work", name="vrow12")
            nc.scalar.activation(vrow[0:1, 0:800], rs12ps[:], AF.Ln,
                                 bias=bZero[0:1, :], scale=1.0)
            nc.vector.tensor_scalar(vrow[0:1, 0:800], vrow[0:1, 0:800], -1.0,
                                    -C12 - LNT, op0=ALU.mult, op1=ALU.add)
            nc.sync.dma_start(b12d.ap(), vrow[0:1, 0:800])
            nc.sync.dma_start(bias12[:, 0:6], AP(b12d, 0, [[1, 128], [128, 6]]))
            nc.sync.dma_start(bias12[0:32, 6:7], AP(b12d, 768, [[1, 32], [32, 1]]))

            # sim14^T: colsum only
            for sc in range(50):
                ps = psA.tile([128, 800], F32, tag="p1ps")
                te = texp_p.tile([128, 800], BF16, tag="texp")
                lhs_h = s14_1h[:, sc * 128:(sc + 1) * 128]
                lhs_l = s14_1l[0:69, sc * 128:(sc + 1) * 128]
                for c0, c1 in [(0, 512), (512, 800)]:
                    nc.tensor.matmul(ps[:, c0:c1], lhs_h, s14_0h[:, c0:c1],
                                     start=True, stop=False)
                    nc.tensor.matmul(ps[:, c0:c1], lhs_l, s14_0l[0:69, c0:c1],
                                     start=False, stop=True)
                nc.scalar.activation(te[:], ps[:], AF.Exp, bias=bC14[:],
                                     scale=1.0, accum_out=cs14[:, sc:sc + 1])

            # sim18^T [s(<=128), l(256 pad)]
            rs18ps = psB.tile([1, 800], F32, tag="rsB")
            for sc in range(13):
                kw = 128 if sc < 12 else 64
                ps = psA.tile([128, 800], F32, tag="p1ps")
                te = texp_p.tile([128, 800], BF16, tag="texp")
                nc.tensor.matmul(ps[0:kw, 0:256],
                                 s18_1h[:, sc * 128:sc * 128 + kw],
                                 s18_0h[:], start=True, stop=False)
                nc.tensor.matmul(ps[0:kw, 0:256],
                                 s18_1l[:, sc * 128:sc * 128 + kw],
                                 s18_0l[:], start=False, stop=True)
                nc.scalar.activation(te[0:kw, 0:256], ps[0:kw, 0:256], AF.Exp,
                                     bias=bC18[0:kw, :], scale=1.0,
                                     accum_out=cs18[0:kw, sc:sc + 1])
                nc.tensor.matmul(rs18ps[:, 0:256], sob[0:kw, :],
                                 te[0:kw, 0:256],
                                 start=(sc == 0), stop=(sc == 12))
            vrow18 = work.tile([128, 1600], F32, tag="work", name="vrow18")
            nc.scalar.activation(vrow18[0:1, 0:256], rs18ps[:, 0:256], AF.Ln,
                                 bias=bZero[0:1, :], scale=1.0)
            nc.vector.tensor_scalar(vrow18[0:1, 0:256], vrow18[0:1, 0:256],
                                    -1.0, -C18, op0=ALU.mult, op1=ALU.add)
            nc.sync.dma_start(b18d.ap(), vrow18[0:1, 0:256])
            nc.sync.dma_start(bias18[:, 0:1], AP(b18d, 0, [[1, 128], [128, 1]]))
            nc.sync.dma_start(bias18[0:72, 1:2], AP(b18d, 128, [[1, 72], [72, 1]]))

            # sim116^T (replicated, local colsum only, no collective)
            for sc in range(4):
                kw = 128 if sc < 3 else 16
                ps = psA.tile([128, 800], F32, tag="p1ps")
                te = texp_p.tile([128, 800], BF16, tag="texp")
                nc.tensor.matmul(ps[0:kw, 0:400],
                                 s116_1h[:, sc * 128:sc * 128 + kw],
                                 s116_0h[:], start=True, stop=False)
                nc.tensor.matmul(ps[0:kw, 0:400],
                                 s116_1l[:, sc * 128:sc * 128 + kw],
                                 s116_0l[:], start=False, stop=True)
                nc.scalar.activation(te[0:kw, 0:400], ps[0:kw, 0:400], AF.Exp,
                                     bias=bC116[0:kw, :], scale=1.0,
                                     accum_out=cs116[0:kw, sc:sc + 1])

            # ---- aug116 row (local; no collective) ----
            nc.scalar.activation(lnst[:, 0:4], cs116[:], AF.Ln, bias=bZero[:],
                                 scale=1.0)
            nc.vector.tensor_scalar(augst[:, 0:4], lnst[:, 0:4], -0.5,
                                    -C116 / 2.0, op0=ALU.mult, op1=ALU.add)
            nc.sync.dma_start(AP(aug116d, 0, [[1, 128], [128, 4]]),
                              augst[:, 0:4])
            gp.dma_start(augrow116, aug116d.ap().rearrange(
                "(o f) -> o f", o=1))

            # ============ P2-116: argmax + gather + conv + t2p ============
            for lc in range(4):
                kw = 128 if lc < 3 else 16
                ps = psA.tile([128, 800], F32, tag="p1ps")
                nc.tensor.matmul(ps[0:kw, 0:400],
                                 s116_0h[:, lc * 128:lc * 128 + kw],
                                 s116_1h[:], start=True, stop=False)
                nc.tensor.matmul(ps[0:kw, 0:400],
                                 s116_0l[:, lc * 128:lc * 128 + kw],
                                 s116_1l[:], start=False, stop=False)
                nc.tensor.matmul(ps[0:kw, 0:400], ones96t[96:97, 0:kw],
                                 augrow116[:, 0:400], start=False, stop=True,
                                 tile_position=(96, 0))
                z = work.tile([128, 1600], F32, tag="work")
                nc.scalar.copy(z[0:kw, 0:400], ps[0:kw, 0:400])
                nc.vector.max(mx8[0:kw, :], z[0:kw, 0:400])
                nc.vector.max_index(mi8_16[0:kw, :], mx8[0:kw, :],
                                    z[0:kw, 0:400])
                nc.vector.tensor_copy(idx116[0:kw, lc:lc + 1],
                                      mi8_16[0:kw, 0:1])
            nc.sync.dma_start(AP(i116d, 0, [[1, 128], [128, 4]]),
                              idx116[:].bitcast(I16))
            for g in range(8):
                nc.sync.dma_start(idx116w[16 * g:16 * g + 16, :],
                                  AP(i116d, 0, [[1, 16], [16, 25]]))
            for i, src in enumerate([s116_1h, s116_1l]):
                gp.ap_gather(f116gr[i][:, 0:400].rearrange(
                                 "p (n d) -> p n d", d=1),
                             src[:].bitcast(F32).rearrange(
                                 "p (n d) -> p n d", d=1),
                             idx116w[:, :], channels=128, num_elems=400, d=1,
                             num_idxs=400)
            conv116_rhs = [st2h[:].bitcast(F32), st2l[:].bitcast(F32),
                           s116_0h[:].bitcast(F32), s116_0l[:].bitcast(F32),
                           f116gr[0][:, 0:400], f116gr[1][:, 0:400]]
            for ob in range(2):
                psc = psA.tile([128, 800], F32, tag="p1ps")
                for k in range(6):
                    w = wpool.tile([128, 128], F32, tag="w")
                    nc.sync.dma_start(
                        w[:], W116T[k * 128:(k + 1) * 128,
                                    ob * 128:(ob + 1) * 128])
                    nc.tensor.matmul(psc[:, 0:400], w[:], conv116_rhs[k],
                                     start=(k == 0), stop=(k == 5))
                # pool 2x2 on 20x20 -> write [128,100] into zeroed 256 cols
                t2full = work.tile([128, 1600], F32, tag="work", name="t2full")
                nc.scalar.copy(t2full[:, 0:400], psc[:, 0:400])
                nc.vector.memset(t2pr[ob][:], 0.0)
                nc.vector.tensor_reduce(
                    t2pr[ob][:, 0:100].rearrange("p (i j) -> p i j", i=10),
                    t2full[:, 0:400].rearrange(
                        "p (i a j b) -> p i j a b", i=10, a=2, j=10, b=2),
                    axis=mybir.AxisListType.XY, op=ALU.add)

            # ============ CC1: AllReduce column sums ============
            nc.sync.dma_start(AP(cc1_in.tensor, 0, [[1, 128], [128, 50]]),
                              cs12[:])
            nc.sync.dma_start(AP(cc1_in.tensor, 6400, [[1, 128], [128, 50]]),
                              cs14[:])
            nc.sync.dma_start(AP(cc1_in.tensor, 12800, [[1, 128], [128, 13]]),
                              cs18[:])
            gp.collective_compute("AllReduce", ALU.add, replica_groups=RG,
                                  ins=[cc1_in[:].opt()], outs=[cc1_out[:].opt()])

            # ---- post-cc1: aug rows for 12 / 14 / 18 ----
            for off, n_col, cc, dram_s, dst_ap in [
                (0, 50, C12, aug12d, augrow12),
                (6400, 50, C14, aug14d, s14_1l[68:69, 0:6400]),
                (12800, 13, C18, aug18d, augrow18),  # cols 6400:8064
            ]:
                csr = work.tile([128, 1600], F32, tag="work")
                nc.sync.dma_start(csr[:, 0:n_col],
                                  AP(cc1_out.tensor, off, [[1, 128], [128, n_col]]))
                nc.scalar.activation(lnst[:, 0:n_col], csr[:, 0:n_col], AF.Ln,
                                     bias=bZero[:], scale=1.0)
                nc.vector.tensor_scalar(augst[:, 0:n_col], lnst[:, 0:n_col],
                                        -0.5, -cc / 2.0, op0=ALU.mult,
                                        op1=ALU.add)
                nc.sync.dma_start(AP(dram_s, 0, [[1, 128], [128, n_col]]),
                                  augst[:, 0:n_col])
                gp.dma_start(dst_ap, dram_s.ap().rearrange("(o f) -> o f", o=1)
                             [0:1, 0:dst_ap.shape[-1]])

            # ============ P2-14: argmax rows ============
            for lc in range(7):
                kw = 128 if lc < 6 else 32
                z14 = big.tile([128, 6400], F32, tag="big")
                for scc in range(13):
                    ps = psA.tile([128, 800], F32, tag="p1ps")
                    s0 = scc * 512
                    s1 = min(s0 + 512, 6400)
                    w = s1 - s0
                    nc.tensor.matmul(ps[0:kw, 0:w],
                                     s14_0h[:, lc * 128:lc * 128 + kw],
                                     s14_1h[:, s0:s1], start=True, stop=False)
                    nc.tensor.matmul(ps[0:kw, 0:w],
                                     s14_0l[0:69, lc * 128:lc * 128 + kw],
                                     s14_1l[0:69, s0:s1], start=False,
                                     stop=True)
                    nc.scalar.copy(z14[0:kw, s0:s1], ps[0:kw, 0:w])
                nc.vector.max(mx8[0:kw, :], z14[0:kw, :])
                nc.vector.max_index(mi8_16[0:kw, :], mx8[0:kw, :], z14[0:kw, :])
                nc.vector.tensor_copy(idx14[0:kw, lc:lc + 1], mi8_16[0:kw, 0:1])
            nc.sync.dma_start(AP(i14d, 0, [[1, 128], [128, 7]]),
                              idx14[:].bitcast(I16))
            for g in range(8):
                nc.sync.dma_start(idx14w[16 * g:16 * g + 16, :],
                                  AP(i14d, 0, [[1, 16], [16, 50]]))
            f14g_h = work.tile([128, 1600], F32, tag="work", name="f14g_h")
            f14g_l = work.tile([128, 1600], F32, tag="work", name="f14g_l")
            for src, dstf, ch in [(s14_1h, f14g_h, 128), (s14_1l, f14g_l, 80)]:
                gp.ap_gather(dstf[0:ch, 0:800].rearrange("p (n d) -> p n d", d=1),
                             src[0:ch, :].bitcast(F32).rearrange(
                                 "p (n d) -> p n d", d=1),
                             idx14w[0:ch, :], channels=ch, num_elems=6400,
                             d=1, num_idxs=800)
            # rowbands (sum over b=8) -> cc2[0:39200]
            rb_srcs = [(s14_0h[:].bitcast(F32), 128, 0),
                       (s14_0l[0:68, :].bitcast(F32), 68, 12800),
                       (f14g_h[:, 0:800], 128, 19600),
                       (f14g_l[0:68, 0:800], 68, 32400)]
            for src_ap, ch, off in rb_srcs:
                nc.vector.tensor_reduce(
                    rbst[0:ch, 0:100].rearrange("p (a j) -> p a j", a=10),
                    src_ap.rearrange("p (a j b) -> p a j b", a=10, j=10, b=8),
                    axis=mybir.AxisListType.X, op=ALU.add)
                nc.sync.dma_start(AP(cc2_in.tensor, off, [[100, ch], [1, 100]]),
                                  rbst[0:ch, 0:100])

            # ============ P2-18: conf18 out + argmax + conv ============
            for lc in range(2):
                kw = 128 if lc == 0 else 72
                c18 = work.tile([128, 1600], F32, tag="work", name="c18")
                for h0, h1 in [(0, 800), (800, 1600)]:
                    ps = psA.tile([128, 800], F32, tag="p1ps", name="ps18")
                    for s0, s1 in [(h0, h0 + 512), (h0 + 512, h1)]:
                        r0, r1 = s0 - h0, s1 - h0
                        nc.tensor.matmul(ps[0:kw, r0:r1],
                                         s18_0h[:, lc * 128:lc * 128 + kw],
                                         s18_1h[:, s0:s1], start=True,
                                         stop=False)
                        nc.tensor.matmul(ps[0:kw, r0:r1],
                                         s18_0l[:, lc * 128:lc * 128 + kw],
                                         s18_1l[:, s0:s1], start=False,
                                         stop=False)
                        nc.tensor.matmul(ps[0:kw, r0:r1], ones96t[96:97, 0:kw],
                                         augrow18[:, s0:s1], start=False,
                                         stop=True, tile_position=(96, 0))
                    nc.scalar.activation(c18[0:kw, h0:h1], ps[0:kw, 0:800],
                                         AF.Exp, bias=bias18[0:kw, lc:lc + 1],
                                         scale=2.0)
                nc.sync.dma_start(conf18_o[lc * 128:lc * 128 + kw, :],
                                  c18[0:kw, :])
                nc.vector.max(mx8[0:kw, :], c18[0:kw, :])
                nc.vector.max_index(mi8_16[0:kw, :], mx8[0:kw, :], c18[0:kw, :])
                nc.vector.tensor_copy(idx18[0:kw, lc:lc + 1], mi8_16[0:kw, 0:1])
            nc.sync.dma_start(AP(i18d, 0, [[1, 128], [128, 2]]),
                              idx18[:].bitcast(I16))
            for g in range(8):
                nc.sync.dma_start(idx18w[16 * g:16 * g + 16, :],
                                  AP(i18d, 0, [[1, 16], [16, 13]]))
            for i, src in enumerate([s18_1h, s18_1l]):
                nc.vector.memset(f18gr[i][:], 0.0)
                gp.ap_gather(f18gr[i][:, 0:200].rearrange(
                                 "p (n d) -> p n d", d=1),
                             src[:].bitcast(F32).rearrange(
                                 "p (n d) -> p n d", d=1),
                             idx18w[:, :], channels=128, num_elems=1600, d=1,
                             num_idxs=200)
            conv18_rhs = [s18_0h[:].bitcast(F32), s18_0l[:].bitcast(F32),
                          f18gr[0][:], f18gr[1][:]]
            for ob in range(2):
                psc = psA.tile([128, 800], F32, tag="p1ps")
                for k in range(4):
                    w = wpool.tile([128, 128], F32, tag="w")
                    nc.sync.dma_start(
                        w[:], W18T[k * 128:(k + 1) * 128,
                                   ob * 128:(ob + 1) * 128])
                    nc.tensor.matmul(psc[:, 0:256], w[:], conv18_rhs[k],
                                     start=(k == 0), stop=(k == 3))
                c18c = work.tile([128, 1600], F32, tag="work", name="c18cf")
                nc.scalar.copy(c18c[:, 0:256], psc[:, 0:256])
                nc.vector.tensor_reduce(
                    rbst[:, 0:50].rearrange("p (a j) -> p a j", a=5),
                    c18c[:, 0:200].rearrange(
                        "p (a j b) -> p a j b", a=5, j=10, b=4),
                    axis=mybir.AxisListType.X, op=ALU.add)
                nc.sync.dma_start(
                    AP(cc2_in.tensor, 39200 + ob * 6400, [[50, 128], [1, 50]]),
                    rbst[:, 0:50])

            # ============ CC2: AllGather rowbands ============
            gp.collective_compute("AllGather", ALU.bypass, replica_groups=RG,
                                  ins=[cc2_in[:].opt()], outs=[cc2_out[:].opt()])

            # ---- post-cc2: global pools ----
            # f14 segs: (offset, channels, pool-slot)
            f14segs = [(0, 128, 0), (12800, 68, 1), (19600, 128, 2),
                       (32400, 68, 3)]
            for off, ch, slot in f14segs:
                g = work.tile([128, 1600], F32, tag="work", name="g14")
                nc.sync.dma_start(
                    g[0:ch, 0:800],
                    AP(cc2_out.tensor, off, [[100, ch], [52000, 8], [1, 100]]))
                nc.vector.memset(poolr[slot][:], 0.0)
                nc.vector.tensor_reduce(
                    poolr[slot][0:ch, 0:100].rearrange(
                        "p (i j) -> p i j", i=10),
                    g[0:ch, 0:800].rearrange(
                        "p (i a j) -> p i j a", i=10, a=8, j=10),
                    axis=mybir.AxisListType.X, op=ALU.add)
            for ob in range(2):
                g = work.tile([128, 1600], F32, tag="work", name="g18")
                nc.sync.dma_start(
                    g[:, 0:400],
                    AP(cc2_out.tensor, 39200 + ob * 6400,
                       [[50, 128], [52000, 8], [1, 50]]))
                slot = 4 + ob
                nc.vector.memset(poolr[slot][:], 0.0)
                nc.vector.tensor_reduce(
                    poolr[slot][:, 0:100].rearrange(
                        "p (i j) -> p i j", i=10),
                    g[:, 0:400].rearrange(
                        "p (i a j) -> p i j a", i=10, a=4, j=10),
                    axis=mybir.AxisListType.X, op=ALU.add)

            # ---- t3 conv -> t3d -> t4 conv -> t4seq ----
            t3_rhs = [(st3h[:], 128, 0), (st3l[:], 68, 128),
                      (t2pr[0][:], 128, 196), (t2pr[1][:], 128, 324),
                      (poolr[4][:], 128, 452), (poolr[5][:], 128, 580),
                      (poolr[0][:], 128, 708), (poolr[1][0:68, :], 68, 836),
                      (poolr[2][:], 128, 904), (poolr[3][0:68, :], 68, 1032)]
            pst3 = psA.tile([128, 800], F32, tag="p1ps")
            for k, (rhs, kw, roff) in enumerate(t3_rhs):
                w = wpool.tile([128, 128], F32, tag="w")
                nc.sync.dma_start(w[0:kw, :], Wt3T[roff:roff + kw, :])
                nc.tensor.matmul(pst3[:, 0:256], w[0:kw, :], rhs,
                                 start=(k == 0), stop=(k == 9))
            t3cf = work.tile([128, 1600], F32, tag="work", name="t3cf")
            nc.scalar.copy(t3cf[:, 0:256], pst3[:, 0:256])
            nc.vector.memset(t3df[:], 0.0)
            nc.vector.tensor_reduce(
                t3df[:, 0:25].rearrange("p (i j) -> p i j", i=5),
                t3cf[:, 0:100].rearrange(
                    "p (i a j b) -> p i j a b", i=5, a=2, j=5, b=2),
                axis=mybir.AxisListType.XY, op=ALU.add)
            pst4 = psA.tile([128, 800], F32, tag="p1ps")
            w4a = wpool.tile([128, 128], F32, tag="w")
            w4b = wpool.tile([128, 128], F32, tag="w")
            nc.sync.dma_start(w4a[:], Wt4T[0:128, :])
            nc.sync.dma_start(w4b[:], Wt4T[128:256, :])
            nc.tensor.matmul(pst4[:, 0:32], w4a[:], t3df[:], start=True,
                             stop=False)
            nc.tensor.matmul(pst4[:, 0:32], w4b[:], st4[:], start=False,
                             stop=True)
            nc.scalar.copy(t4sf[:], pst4[:, 0:32])

            # ---- dmatrix + topic argmax ----
            for lc in range(13):
                kw = 128 if lc < 12 else 64
                ps = psA.tile([128, 800], F32, tag="p1ps")
                nc.tensor.matmul(ps[0:kw, 0:32],
                                 s12cat[:, lc * 128:lc * 128 + kw]
                                 .bitcast(F32),
                                 t4sf[:], start=True, stop=True)
                z = work.tile([128, 1600], F32, tag="work")
                nc.scalar.copy(z[0:kw, 0:32], ps[0:kw, 0:32])
                nc.vector.max(mx8[0:kw, :], z[0:kw, 0:25])
                nc.vector.max_index(mi8_32[0:kw, :], mx8[0:kw, :],
                                    z[0:kw, 0:25])
                nc.vector.tensor_copy(topicacc[0:kw, lc:lc + 1],
                                      mi8_32[0:kw, 0:1])
            nc.sync.dma_start(AP(topic_o, 0, [[1, 128], [128, 12]]),
                              topicacc[:, 0:12].bitcast(I32))
            nc.sync.dma_start(AP(topic_o, 1536, [[1, 64], [64, 1]]),
                              topicacc[0:64, 12:13].bitcast(I32))

            # ============ P2-12: conf12 ============
            for lc in range(7):
                kw = 128 if lc < 6 else 32
                val = big.tile([128, 6400], F32, tag="big")
                for scc in range(13):
                    ps = psA.tile([128, 800], F32, tag="p1ps")
                    s0 = scc * 512
                    s1 = min(s0 + 512, 6400)
                    w = s1 - s0
                    nc.tensor.matmul(ps[0:kw, 0:w],
                                     s12cat[:, lc * 128:lc * 128 + kw],
                                     s12_1[:, s0:s1], start=True, stop=False)
                    nc.tensor.matmul(ps[0:kw, 0:w], ones96t[96:97, 0:kw],
                                     augrow12[:, s0:s1], start=False, stop=True,
                                     tile_position=(96, 0))
                    nc.scalar.activation(val[0:kw, s0:s1], ps[0:kw, 0:w],
                                         AF.Exp, bias=bias12[0:kw, lc:lc + 1],
                                         scale=2.0)
                    nc.vector.scalar_tensor_tensor(
                        val[0:kw, s0:s1], val[0:kw, s0:s1], 1.0,
                        val[0:kw, s0:s1], op0=ALU.is_gt, op1=ALU.mult)
                nc.sync.dma_start(conf12_o[lc * 128:lc * 128 + kw, :],
                                  val[0:kw, :])
    nc.compile()
    return nc


def _get_nc():
    global _nc_cache
    if _nc_cache is None:
        _nc_cache = _build_nc()
    return _nc_cache


def _host_prep(inputs):
    import ml_dtypes
    f32 = np.float32

    def T(x):
        return np.ascontiguousarray(x.T, dtype=f32)

    f12_0 = np.asarray(inputs['feats12_0'][0], f32)
    f12_1 = np.asarray(inputs['feats12_1'][0], f32)
    f14_0 = np.asarray(inputs['feats14_0'][0], f32)
    f14_1 = np.asarray(inputs['feats14_1'][0], f32)
    f18_0 = np.asarray(inputs['feats18_0'][0], f32)
    f18_1 = np.asarray(inputs['feats18_1'][0], f32)
    f116_0 = np.asarray(inputs['feats116_0'][0], f32)
    f116_1 = np.asarray(inputs['feats116_1'][0], f32)
    t2 = np.asarray(inputs['t2'][0], f32)
    t3 = np.asarray(inputs['t3'][0], f32)
    t4 = np.asarray(inputs['t4'][0], f32)
    W116 = np.asarray(inputs['W_down1_16'], f32)
    W18 = np.asarray(inputs['W_down1_8'], f32)
    Wt3 = np.asarray(inputs['W_downt3'], f32)
    Wt4 = np.asarray(inputs['W_downt4'], f32)

    f12_1T = T(f12_1)
    f14_1T = T(f14_1)
    f14_1T_hi = np.ascontiguousarray(f14_1T[0:128])
    f14_1T_lo = np.zeros((128, 8576), f32)
    f14_1T_lo[0:68, 0:6400] = f14_1T[128:196]
    f18_1T = T(f18_1)
    f116_0T = T(f116_0)
    f116_1T = T(f116_1)
    t2T = T(t2)
    t3T = T(t3)
    t3T_hi = np.zeros((128, 256), f32)
    t3T_hi[:, 0:100] = t3T[0:128]
    t3T_lo = np.zeros((68, 256), f32)
    t3T_lo[:, 0:100] = t3T[128:196]
    Wt3T = T(Wt3)  # [1100, 128]
    Wt3T[196:452] *= 0.25      # t2p pool 2x2
    Wt3T[452:708] *= 1.0 / 16  # feats18 pool 4x4
    Wt3T[708:1100] *= 1.0 / 64  # feats14 pool 8x8
    Wt4T = T(Wt4)  # [256, 128]
    Wt4T[0:128] *= 0.25        # t3d pool 2x2

    shared = {
        'f12_1T': f12_1T,
        'f14_1T_hi': f14_1T_hi, 'f14_1T_lo': f14_1T_lo,
        'f18_1T_hi': np.ascontiguousarray(f18_1T[0:128]),
        'f18_1T_lo': np.ascontiguousarray(f18_1T[128:256]),
        'f116_0T_hi': np.ascontiguousarray(f116_0T[0:128]),
        'f116_0T_lo': np.ascontiguousarray(f116_0T[128:256]),
        'f116_1T_hi': np.ascontiguousarray(f116_1T[0:128]),
        'f116_1T_lo': np.ascontiguousarray(f116_1T[128:256]),
        't2T_hi': np.ascontiguousarray(t2T[0:128]),
        't2T_lo': np.ascontiguousarray(t2T[128:256]),
        't3T_hi': t3T_hi, 't3T_lo': t3T_lo,
        't4T': np.concatenate([T(t4), np.zeros((128, 7), f32)], axis=1),
        'W116T': T(W116), 'W18T': T(W18), 'Wt3T': Wt3T, 'Wt4T': Wt4T,
        'ones1r': np.ones((1, 128), f32),
        'onesb': np.ones((128, 1), ml_dtypes.bfloat16),
    }
    in_maps = []
    for r in range(N_CORES):
        m = dict(shared)
        r12 = slice(800 * r, 800 * r + 800)
        r18 = slice(200 * r, 200 * r + 200)
        f12_0T = T(f12_0[r12])
        cat = np.empty((128, 1600), f32)
        cat[:, 0:800] = f12_0T
        cat[:, 800:1600] = f12_1T[:, r12]
        m['f12catT'] = cat
        f14_0T = T(f14_0[r12])
        m['f14_0T_hi'] = np.ascontiguousarray(f14_0T[0:128])
        lo = np.zeros((80, 800), f32)
        lo[0:68] = f14_0T[128:196]
        lo[68] = 1.0
        m['f14_0T_lo'] = lo
        f18_0T = T(f18_0[r18])
        hi = np.zeros((128, 256), f32)
        hi[:, 0:200] = f18_0T[0:128]
        lo8 = np.zeros((128, 256), f32)
        lo8[:, 0:200] = f18_0T[128:256]
        m['f18_0T_hi'] = hi
        m['f18_0T_lo'] = lo8
        in_maps.append(m)
    return in_maps


def kernel(**inputs):
    from concourse.bass_utils import run_bass_kernel_spmd
    nc = _get_nc()
    in_maps = _host_prep(inputs)
    res = run_bass_kernel_spmd(nc, in_maps, core_ids=list(range(N_CORES)))
    conf18 = np.empty((1600, 1600), np.float32)
    conf12 = np.empty((6400, 6400), np.float32)
    topic = np.empty((12800,), np.int32)
    for r in range(N_CORES):
        o = res.results[r]
        conf18[200 * r:200 * r + 200] = o['conf18_o']
        conf12[800 * r:800 * r + 800] = o['conf12_o']
        t = o['topic_o']
        topic[800 * r:800 * r + 800] = t[0:800]
        topic[6400 + 800 * r:6400 + 800 * r + 800] = t[800:1600]
    return conf18[None], conf12[None], topic[None]
